# revision 40
# baseline (speedup 1.0000x reference)
"""MLA (multi-head latent attention) Trainium2 kernel, 8-way tensor/data parallel.

Problem shapes (hardcoded): B=2, S=2048, HID=2048, H=16, KVH=4, DH=128, L=64.

Sharding: core c -> batch b = c//4, kv-group g = c%4.
Each core computes q-heads 4g..4g+3 and kv head g for its batch:
  - q/k/v projections (bf16 operands, f32 PSUM accum), RoPE
  - k-major attention (scoresT [Sk,Sq]) with no-max softmax, denominator via
    ones-matmul on PE, normalization via gpsimd partition-broadcast of 1/den
  - AllGather of per-head attnT (bf16) within the 4-core batch group
  - column-sharded output projection (each core owns 512 output columns)
  - latent cross-attention branch; per-kv-head contribution ReduceScattered so
    each core receives exactly its 512-column slice, folded into the output
    matmul as a rank-1 update.

Scheduling notes:
  - a dummy barrier collective at kernel start absorbs inter-core launch skew
    while the input DMAs stream, so the real collectives don't inherit it
  - hsT is staged seq-quarter-major so input DMAs move 4KB-contiguous rows
  - stage B runs two q-blocks in lockstep (interleaved per k-chunk) so each
    engine works on one stream while the other stream's cross-engine
    semaphore round-trip is in flight
  - stage C runs in two passes so the heads-0..2 contraction hides under the
    last AllGather; per-iteration serial tails never touch the Tensor queue.
Host side shards/prepares inputs and concatenates the 8 per-core outputs.
"""

import numpy as np
import ml_dtypes
from collections import deque
from contextlib import ExitStack

import concourse.bass as bass
import concourse.bacc as bacc
import concourse.tile as tile
import concourse.mybir as mybir
from concourse.bass_utils import run_bass_kernel_spmd

BF16 = ml_dtypes.bfloat16
FP32 = mybir.dt.float32
BF16_DT = mybir.dt.bfloat16

B, S, HID = 2, 2048, 2048
H, KVH, DH, L = 16, 4, 128, 64
THETA = 10000.0
N_CORES = 8
GROUPS = [[0, 1, 2, 3], [4, 5, 6, 7]]
NJ = HID // 128          # 16 contraction chunks
NSQ = S // 512           # 4 seq blocks of 512
NSB = S // 128           # 16 seq blocks of 128
QW = NJ * 512            # 8192 cols per seq-quarter in the sq-major hsT
SCALE = 1.0 / float(np.sqrt(np.float32(DH)))

_COMPILED = {}


def _emit_body(nc, tc, ctx, d, single_core):
    """Emit one full forward pass. d: dict of dram tensor handles."""
    # ---------- persistent pools ----------
    qk_pool = ctx.enter_context(tc.tile_pool(name="qk", bufs=1))
    v_pool = ctx.enter_context(tc.tile_pool(name="v", bufs=1))
    attn_pool = ctx.enter_context(tc.tile_pool(name="attn", bufs=1))
    const_pool = ctx.enter_context(tc.tile_pool(name="const", bufs=1))
    dram_pool = ctx.enter_context(tc.tile_pool(name="dram", bufs=1, space="DRAM"))

    # qT for 4 heads + kT (all roped, bf16):  [128, 5*S]
    qkT = qk_pool.tile([128, 5 * S], BF16_DT)
    # v in [s-part, dh] block layout: block sb -> [:, sb*128:(sb+1)*128]
    v_sb = v_pool.tile([128, S], BF16_DT)
    # normalized attnT for local 4 heads
    attnT = attn_pool.tile([128, 4 * S], BF16_DT)

    ones_col = const_pool.tile([128, 1], BF16_DT)     # den lhsT
    ones_row = const_pool.tile([1, 128], FP32)        # rank1 lhsT
    ident = const_pool.tile([128, 128], BF16_DT)
    shmat = const_pool.tile([128, 128], BF16_DT)      # rope half-swap permut.
    nc.vector.memset(ones_col[:], 1.0)
    nc.vector.memset(ones_row[:], 1.0)
    nc.sync.dma_start(ident[:], d["ident"].ap())
    nc.sync.dma_start(shmat[:], d["shmat"].ap())

    # DRAM bounce buffers for collectives (per-head AllGathers pipeline)
    bar_in = dram_pool.tile([1, 64], BF16_DT, name="bar_in")
    bar_out = dram_pool.tile([4, 64], BF16_DT, name="bar_out")
    ag_in = [dram_pool.tile([128, S], BF16_DT, tag=f"agi{h}", name=f"ag_in{h}")
             for h in range(4)]
    ag_out = [dram_pool.tile([512, S], BF16_DT, tag=f"ago{h}", name=f"ag_out{h}")
              for h in range(4)]
    rs_in = dram_pool.tile([1, HID], FP32)
    rs_out = dram_pool.tile([1, 512], FP32)

    # launch-skew barrier: enqueue first so it soaks up core start skew
    # while the input DMAs stream in
    if not single_core:
        nc.gpsimd.collective_compute(
            "AllGather", mybir.AluOpType.bypass, replica_groups=GROUPS,
            ins=[bar_in.opt()], outs=[bar_out.opt()])

    # ---------- stage A: projections + rope ----------
    with ExitStack() as actx:
        hs_pool = actx.enter_context(tc.tile_pool(name="hs", bufs=1))
        w_pool = actx.enter_context(tc.tile_pool(name="w", bufs=1))
        trig_pool = actx.enter_context(tc.tile_pool(name="trig", bufs=1))
        rope_pool = actx.enter_context(tc.tile_pool(name="rope", bufs=5))
        pA = actx.enter_context(tc.tile_pool(name="pA", bufs=2, space="PSUM"))
        pSh = actx.enter_context(tc.tile_pool(name="pSh", bufs=1, space="PSUM"))
        pT = actx.enter_context(tc.tile_pool(name="pT", bufs=1, space="PSUM"))

        # small tensors first so the first projections aren't DMA-starved
        wq_sb = w_pool.tile([128, NJ * 512], BF16_DT)
        wk_sb = w_pool.tile([128, NJ * 128], BF16_DT)
        wv_sb = w_pool.tile([128, NJ * 128], BF16_DT)
        nc.sync.dma_start(wk_sb[:], d["wk_sb"].ap())
        nc.sync.dma_start(wv_sb[:], d["wv_sb"].ap())
        nc.sync.dma_start(wq_sb[:], d["wq_sb"].ap())

        cosT = trig_pool.tile([128, S], FP32)
        sinS = trig_pool.tile([128, S], FP32)
        nc.sync.dma_start(cosT[:], d["cosT"].ap())
        nc.sync.dma_start(sinS[:], d["sinS"].ap())

        # hsT staged seq-quarter-major: quarter sq occupies cols [sq*QW,
        # (sq+1)*QW) with contraction chunk j at [sq*QW + j*512, ...).
        # 4KB-contiguous rows per DMA descriptor.
        hsT = hs_pool.tile([128, NSQ * QW], BF16_DT)
        for sq in range(NSQ):
            for c in range(4):
                nc.sync.dma_start(
                    hsT[:, sq * QW + c * 2048: sq * QW + (c + 1) * 2048],
                    d["hsT"][sq][:, c * 2048:(c + 1) * 2048])

        pending = deque()   # deferred per-block tails (emitted 1 block late)

        def drain(keep):
            while len(pending) > keep:
                pending.popleft()()

        def project_mm(w_sb, w_off, w_stride, sq):
            ps = pA.tile([128, 512], FP32, tag="proj")
            for j in range(NJ):
                nc.tensor.matmul(
                    ps[:],
                    lhsT=w_sb[:, w_off + j * w_stride:
                              w_off + j * w_stride + 128],
                    rhs=hsT[:, sq * QW + j * 512: sq * QW + j * 512 + 512],
                    start=(j == 0), stop=(j == NJ - 1),
                )
            return ps

        def rope_tail(ps, dst, dst_off, sq):
            def tail():
                # rope:  out = ps*cos + shift(ps)*sinS  (sign folded into sinS)
                # the partition half-swap runs on the PE via a permutation
                # matrix so no DMA sits in the chain
                qf = rope_pool.tile([128, 512], BF16_DT, tag="qf")
                nc.scalar.copy(qf[:], ps[:])
                qs = pSh.tile([128, 512], FP32, tag="qs")
                nc.tensor.matmul(qs[:], lhsT=shmat[:], rhs=qf[:],
                                 start=True, stop=True)
                m1 = rope_pool.tile([128, 512], FP32, tag="m1")
                nc.vector.tensor_mul(m1[:], ps[:], cosT[:, bass.ts(sq, 512)])
                m2 = rope_pool.tile([128, 512], FP32, tag="m2")
                nc.vector.tensor_mul(m2[:], qs[:], sinS[:, bass.ts(sq, 512)])
                nc.gpsimd.tensor_add(
                    dst[:, dst_off + sq * 512: dst_off + sq * 512 + 512],
                    m1[:], m2[:])
            return tail

        def v_tail(ps, sq):
            def tail():
                vT_bf = rope_pool.tile([128, 512], BF16_DT, tag="vbf")
                nc.scalar.copy(vT_bf[:], ps[:])
                tp = pT.tile([128, 512], BF16_DT, tag="tp")
                for i in range(4):
                    nc.tensor.transpose(
                        tp[:, bass.ts(i, 128)], vT_bf[:, bass.ts(i, 128)],
                        ident[:])
                nc.vector.tensor_copy(v_sb[:, bass.ts(sq, 512)], tp[:])
            return tail

        # k first, then v, then one q head; latent overlaps remaining q heads
        for sq in range(NSQ):
            ps = project_mm(wk_sb, 0, 128, sq)
            pending.append(rope_tail(ps, qkT, 4 * S, sq))
            drain(1)
        for sq in range(NSQ):
            ps = project_mm(wv_sb, 0, 128, sq)
            pending.append(v_tail(ps, sq))
            drain(1)
        for sq in range(NSQ):
            ps = project_mm(wq_sb, 0, 512, sq)
            pending.append(rope_tail(ps, qkT, 0, sq))
            drain(1)
        drain(0)

        # ---------- latent branch (tiny; overlaps q projections) ----------
        with ExitStack() as lctx:
            l_pool = lctx.enter_context(tc.tile_pool(name="lat", bufs=1))
            pLs = lctx.enter_context(tc.tile_pool(name="pLs", bufs=2, space="PSUM"))
            pLa = lctx.enter_context(tc.tile_pool(name="pLa", bufs=1, space="PSUM"))

            lat0 = l_pool.tile([128, L], BF16_DT)
            wlat = l_pool.tile([128, 128], BF16_DT)
            wlo = l_pool.tile([128, HID], FP32)
            nc.sync.dma_start(lat0[:], d["lat0T"].ap())
            nc.sync.dma_start(wlat[:], d["w_lat"].ap())
            nc.sync.dma_start(wlo[:], d["wlo"].ap())

            lp_ps = pLs.tile([128, L], FP32, tag="scL")
            nc.tensor.matmul(lp_ps[:], lhsT=wlat[:], rhs=lat0[:],
                             start=True, stop=True)
            latpT = l_pool.tile([128, L], BF16_DT)
            nc.scalar.copy(latpT[:], lp_ps[:])

            avL = pLa.tile([128, L], FP32, tag="avL")
            denL = pLa.tile([1, L], FP32, tag="denL")
            for kc4 in range(4):
                scL = pLs.tile([128, 4 * L], FP32, tag="scL")
                for i in range(4):
                    kc = kc4 * 4 + i
                    nc.tensor.matmul(
                        scL[:, bass.ts(i, L)],
                        lhsT=qkT[:, 4 * S + kc * 128: 4 * S + kc * 128 + 128],
                        rhs=latpT[:], start=True, stop=True)
                exL = l_pool.tile([128, 4 * L], BF16_DT, tag="exL")
                nc.scalar.activation(exL[:], scL[:],
                                     mybir.ActivationFunctionType.Exp,
                                     scale=SCALE)
                for i in range(4):
                    kc = kc4 * 4 + i
                    nc.tensor.matmul(avL[:], lhsT=v_sb[:, bass.ts(kc, 128)],
                                     rhs=exL[:, bass.ts(i, L)],
                                     start=(kc == 0), stop=(kc == NSB - 1))
                    nc.tensor.matmul(denL[:], lhsT=ones_col[:],
                                     rhs=exL[:, bass.ts(i, L)],
                                     start=(kc == 0), stop=(kc == NSB - 1))
            recipL = l_pool.tile([1, L], FP32, tag="recipL")
            nc.vector.reciprocal_approx_fast(recipL[:], denL[:])
            bcL_sb = l_pool.tile([128, L], FP32, tag="bcLsb")
            nc.gpsimd.partition_broadcast(bcL_sb[:], recipL[:])
            attLn = l_pool.tile([128, L], FP32, tag="attLn")
            nc.vector.tensor_mul(attLn[:], avL[:], bcL_sb[:])
            # mean over L then project through wlo rows of this kv head
            meanv = l_pool.tile([128, 1], FP32, tag="meanv")
            nc.vector.tensor_reduce(meanv[:], attLn[:],
                                    axis=mybir.AxisListType.X,
                                    op=mybir.AluOpType.add)
            nc.vector.tensor_scalar_mul(meanv[:], meanv[:], 1.0 / L)
            contrib_sb = l_pool.tile([1, HID], FP32, tag="contrib")
            for i in range(4):
                cps = pLs.tile([1, 512], FP32, tag="scL")
                nc.tensor.matmul(cps[:], lhsT=meanv[:],
                                 rhs=wlo[:, bass.ts(i, 512)],
                                 start=True, stop=True)
                nc.vector.tensor_copy(contrib_sb[:, bass.ts(i, 512)], cps[:])
            nc.sync.dma_start(rs_in[:], contrib_sb[:])

        if single_core:
            nc.sync.dma_start(rs_out[:], rs_in[:, 0:512])
        else:
            nc.gpsimd.collective_compute(
                "ReduceScatter", mybir.AluOpType.add, replica_groups=GROUPS,
                ins=[rs_in.opt()], outs=[rs_out.opt()])

        for hh in range(1, 4):
            for sq in range(NSQ):
                ps = project_mm(wq_sb, hh * 128, 512, sq)
                pending.append(rope_tail(ps, qkT, hh * S, sq))
                drain(1)
        drain(0)

    # ---------- stage B + C SBUF pools (hsT freed above) ----------
    with ExitStack() as sctx:
        e_pool = sctx.enter_context(tc.tile_pool(name="expd", bufs=6))
        n_pool = sctx.enter_context(tc.tile_pool(name="norm", bufs=3))
        g_pool = sctx.enter_context(tc.tile_pool(name="gath", bufs=1))
        wo_pool = sctx.enter_context(tc.tile_pool(name="wo", bufs=1))
        oa_pool = sctx.enter_context(tc.tile_pool(name="oacc", bufs=1))
        o_pool = sctx.enter_context(tc.tile_pool(name="oev", bufs=6))

        wo_sb = wo_pool.tile([128, NJ * 512], BF16_DT)
        nc.sync.dma_start(wo_sb[:], d["wo_sb"].ap())
        latrow = wo_pool.tile([1, 512], FP32)
        nc.sync.dma_start(latrow[:], rs_out[:])
        gath = g_pool.tile([128, NJ * S], BF16_DT)
        oacc = oa_pool.tile([128, NSB * 512], BF16_DT)

        # ---------- stage B: attention (k-major, pipelined chunk stream) -----
        # Flat stream of 8 chunk-pairs per (head, q-block) iteration with the
        # score matmuls emitted one chunk ahead of the av/den matmuls, so the
        # exp chain on Scalar never waits for a post-exp Tensor round-trip.
        with ExitStack() as bctx:
            pS = bctx.enter_context(tc.tile_pool(name="pS", bufs=2, space="PSUM"))
            pAV = bctx.enter_context(tc.tile_pool(name="pAV", bufs=2, space="PSUM"))
            pDen = bctx.enter_context(tc.tile_pool(name="pDen", bufs=1, space="PSUM"))
            pCi = bctx.enter_context(tc.tile_pool(name="pCi", bufs=1, space="PSUM"))

            # latent row broadcast once; folded into the first pass-1 group
            latbc = n_pool.tile([128, 512], FP32, tag="latbc")
            nc.gpsimd.partition_broadcast(latbc[:], latrow[:])

            # interleaved output-projection partials for AG groups 0/1:
            # (earliest chunk index, closure) -- popped at <=1 per chunk so a
            # late AllGather can never stall the Tensor queue
            cwork = deque()

            def c_partial(h, sb):
                def work():
                    ps = pCi.tile([128, 512], FP32, tag="ci")
                    js = [4 * r + h for r in range(4)]
                    for idx, j in enumerate(js):
                        nc.tensor.matmul(
                            ps[:],
                            lhsT=gath[:, j * S + sb * 128: j * S + sb * 128 + 128],
                            rhs=wo_sb[:, bass.ts(j, 512)],
                            start=(idx == 0), stop=(idx == 3))
                    if h == 0:
                        nc.vector.tensor_add(oacc[:, bass.ts(sb, 512)],
                                             ps[:], latbc[:])
                    else:
                        nc.vector.tensor_add(oacc[:, bass.ts(sb, 512)],
                                             ps[:], oacc[:, bass.ts(sb, 512)])
                return work

            def attn_tail(hh, sq, av, den):
                recip = n_pool.tile([1, 512], FP32, tag="recip")
                nc.vector.reciprocal_approx_fast(recip[:], den[:])
                bc_sb = n_pool.tile([128, 512], FP32, tag="bcsb")
                nc.gpsimd.partition_broadcast(bc_sb[:], recip[:])
                nc.vector.tensor_mul(
                    attnT[:, hh * S + sq * 512: hh * S + sq * 512 + 512],
                    av[:], bc_sb[:])
                if sq == NSQ - 1:
                    # ship this head's attnT out, gather it across the
                    # group, and pull the gathered rows back into SBUF
                    nc.sync.dma_start(ag_in[hh][:],
                                      attnT[:, hh * S:(hh + 1) * S])
                    if single_core:
                        nc.sync.dma_start(ag_out[hh][0:128, :], ag_in[hh][:])
                    else:
                        nc.gpsimd.collective_compute(
                            "AllGather", mybir.AluOpType.bypass,
                            replica_groups=GROUPS,
                            ins=[ag_in[hh].opt()], outs=[ag_out[hh].opt()])
                    for r in range(4):
                        j = 4 * r + hh
                        nc.sync.dma_start(
                            gath[:, bass.ts(j, S)],
                            ag_out[hh][r * 128:(r + 1) * 128, :])
                    if hh < 2:
                        for sb in range(NSB):
                            cwork.append(((hh + 2) * 32, c_partial(hh, sb)))

            NCH = 16 * 8        # chunk-pairs total (16 iterations x 8)

            def sc_emit(n):
                it, c = divmod(n, 8)
                hh, sq = divmod(it, 4)
                off = hh * S + sq * 512
                sc = pS.tile([128, 1024], FP32, tag="sc")
                for i in range(2):
                    kc = c * 2 + i
                    nc.tensor.matmul(
                        sc[:, bass.ts(i, 512)],
                        lhsT=qkT[:, 4 * S + kc * 128: 4 * S + kc * 128 + 128],
                        rhs=qkT[:, off: off + 512],
                        start=True, stop=True)
                return sc

            scq = deque([sc_emit(0), sc_emit(1)])
            av = den = None
            for n in range(NCH):
                it, c = divmod(n, 8)
                hh, sq = divmod(it, 4)
                if c == 0:
                    av = pAV.tile([128, 512], FP32, tag="av")
                    den = pDen.tile([1, 512], FP32, tag="den")
                sc = scq.popleft()
                ex = e_pool.tile([128, 1024], BF16_DT, tag="ex")
                nc.scalar.activation(ex[:], sc[:],
                                     mybir.ActivationFunctionType.Exp,
                                     scale=SCALE)
                if n + 2 < NCH:
                    scq.append(sc_emit(n + 2))
                for i in range(2):
                    kc = c * 2 + i
                    nc.tensor.matmul(
                        av[:], lhsT=v_sb[:, bass.ts(kc, 128)],
                        rhs=ex[:, bass.ts(i, 512)],
                        start=(kc == 0), stop=(kc == NSB - 1))
                    nc.tensor.matmul(
                        den[:], lhsT=ones_col[:],
                        rhs=ex[:, bass.ts(i, 512)],
                        start=(kc == 0), stop=(kc == NSB - 1))
                if c == 7:
                    attn_tail(hh, sq, av, den)
                if cwork and cwork[0][0] <= n:
                    cwork.popleft()[1]()
            while cwork:
                cwork.popleft()[1]()

        # ---------- stage C: output projection (two passes) ----------
        # pass 1: heads 0..2 contraction, runs under the AG3 collective wait
        with ExitStack() as cctx:
            pC = cctx.enter_context(tc.tile_pool(name="pC", bufs=4, space="PSUM"))

            J012 = [j for j in range(NJ) if j % 4 == 2]
            J3 = [j for j in range(NJ) if j % 4 == 3]
            ctails = deque()
            for sb in range(NSB):
                ps1 = pC.tile([128, 512], FP32, tag="c1")
                for idx, j in enumerate(J012):
                    nc.tensor.matmul(
                        ps1[:],
                        lhsT=gath[:, j * S + sb * 128: j * S + sb * 128 + 128],
                        rhs=wo_sb[:, bass.ts(j, 512)],
                        start=(idx == 0), stop=(idx == len(J012) - 1))

                def c1_tail(sb=sb, ps1=ps1):
                    nc.vector.tensor_add(oacc[:, bass.ts(sb, 512)], ps1[:],
                                         oacc[:, bass.ts(sb, 512)])
                ctails.append(c1_tail)
                while len(ctails) > 1:
                    ctails.popleft()()
            while ctails:
                ctails.popleft()()

            # pass 2: last head's chunks + accumulated pass-1 + latent rank-1
            for sb in range(NSB):
                ps2 = pC.tile([128, 512], FP32, tag="c1")
                for idx, j in enumerate(J3):
                    nc.tensor.matmul(
                        ps2[:],
                        lhsT=gath[:, j * S + sb * 128: j * S + sb * 128 + 128],
                        rhs=wo_sb[:, bass.ts(j, 512)],
                        start=(idx == 0), stop=False)
                nc.tensor.matmul(ps2[:], lhsT=ident[:],
                                 rhs=oacc[:, bass.ts(sb, 512)],
                                 start=False, stop=True)

                def c2_tail(sb=sb, ps2=ps2):
                    oev = o_pool.tile([128, 512], FP32, tag="oev")
                    nc.vector.tensor_copy(oev[:], ps2[:])
                    nc.sync.dma_start(d["y"].ap()[sb * 128:(sb + 1) * 128, :],
                                      oev[:])
                ctails.append(c2_tail)
                while len(ctails) > 1:
                    ctails.popleft()()
            while ctails:
                ctails.popleft()()


def _build_kernel(reps=1, single_core=False):
    nc = bacc.Bacc("TRN2", target_bir_lowering=False, debug=False,
                   num_devices=(1 if single_core else N_CORES))

    d = {
        "hsT": nc.dram_tensor("hsT", [NSQ, 128, QW], BF16_DT, kind="ExternalInput"),
        "wq_sb": nc.dram_tensor("wq_sb", [128, NJ * 512], BF16_DT, kind="ExternalInput"),
        "wk_sb": nc.dram_tensor("wk_sb", [128, NJ * 128], BF16_DT, kind="ExternalInput"),
        "wv_sb": nc.dram_tensor("wv_sb", [128, NJ * 128], BF16_DT, kind="ExternalInput"),
        "wo_sb": nc.dram_tensor("wo_sb", [128, NJ * 512], BF16_DT, kind="ExternalInput"),
        "cosT": nc.dram_tensor("cosT", [128, S], FP32, kind="ExternalInput"),
        "sinS": nc.dram_tensor("sinS", [128, S], FP32, kind="ExternalInput"),
        "ident": nc.dram_tensor("ident", [128, 128], BF16_DT, kind="ExternalInput"),
        "shmat": nc.dram_tensor("shmat", [128, 128], BF16_DT, kind="ExternalInput"),
        "lat0T": nc.dram_tensor("lat0T", [128, L], BF16_DT, kind="ExternalInput"),
        "w_lat": nc.dram_tensor("w_lat", [128, 128], BF16_DT, kind="ExternalInput"),
        "wlo": nc.dram_tensor("wlo", [128, HID], FP32, kind="ExternalInput"),
        "y": nc.dram_tensor("y", [S, 512], FP32, kind="ExternalOutput"),
    }

    with tile.TileContext(nc) as tc:
        if reps == 1:
            with ExitStack() as ctx:
                _emit_body(nc, tc, ctx, d, single_core)
        else:
            with tc.For_i(0, reps, 1):
                with ExitStack() as ctx:
                    _emit_body(nc, tc, ctx, d, single_core)

    nc.compile()
    return nc


def _host_inputs(hs, latent, w_latent, wq, wk, wv, wo, wlo):
    """Build the 8 per-core input maps."""
    inv_freq = 1.0 / (THETA ** (np.arange(0, DH, 2, dtype=np.float32) / DH))
    t = np.arange(S, dtype=np.float32)
    freqs = np.outer(t, inv_freq)
    emb = np.concatenate([freqs, freqs], axis=-1)          # [S, DH]
    cosT = np.ascontiguousarray(np.cos(emb).T.astype(np.float32))
    sinT = np.sin(emb).T.astype(np.float32)
    sinS = sinT.copy()
    sinS[:64] *= -1.0
    sinS = np.ascontiguousarray(sinS)
    ident = np.eye(128, dtype=BF16)
    shmat = np.ascontiguousarray(np.roll(np.eye(128), 64, axis=0)).astype(BF16)

    def chunked(w, cols):
        # [HID, cols] -> [128, NJ*cols] with chunk-major free dim
        return np.ascontiguousarray(
            w.reshape(NJ, 128, cols).transpose(1, 0, 2).reshape(128, NJ * cols)
        ).astype(BF16)

    in_maps = []
    for c in range(N_CORES):
        b, g = c // 4, c % 4
        # [S, HID] -> [NSQ, 128(hid chunk row), NJ*512] seq-quarter-major
        hsT_full = np.ascontiguousarray(hs[b].T).astype(BF16)   # [HID, S]
        hsT = np.empty((NSQ, 128, QW), dtype=BF16)
        for sq in range(NSQ):
            for j in range(NJ):
                hsT[sq, :, j * 512:(j + 1) * 512] = \
                    hsT_full[j * 128:(j + 1) * 128, sq * 512:(sq + 1) * 512]
        in_maps.append({
            "hsT": hsT,
            "wq_sb": chunked(wq[:, 4 * g * DH:(4 * g + 4) * DH], 512),
            "wk_sb": chunked(wk[:, g * DH:(g + 1) * DH], 128),
            "wv_sb": chunked(wv[:, g * DH:(g + 1) * DH], 128),
            "wo_sb": chunked(wo[:, g * 512:(g + 1) * 512], 512),
            "cosT": cosT,
            "sinS": sinS,
            "ident": ident,
            "shmat": shmat,
            "lat0T": np.ascontiguousarray(latent[0, g].T).astype(BF16),
            "w_lat": w_latent.astype(BF16),
            "wlo": np.ascontiguousarray(wlo[g * DH:(g + 1) * DH, :]).astype(np.float32),
        })
    return in_maps


def kernel(hidden_states, latent, w_latent, wq, wk, wv, wo, w_latent_o,
           *, _trace=False, _trace_cores=None):
    hs = np.asarray(hidden_states, np.float32)
    in_maps = _host_inputs(hs, np.asarray(latent), np.asarray(w_latent),
                           np.asarray(wq), np.asarray(wk), np.asarray(wv),
                           np.asarray(wo), np.asarray(w_latent_o))
    if "nc" not in _COMPILED:
        _COMPILED["nc"] = _build_kernel()
    nc = _COMPILED["nc"]
    res = run_bass_kernel_spmd(nc, in_maps, list(range(N_CORES)),
                               trace=_trace, trace_cores=_trace_cores)
    kernel.last_result = res
    out = np.empty((B, S, HID), np.float32)
    for c in range(N_CORES):
        b, g = c // 4, c % 4
        out[b, :, g * 512:(g + 1) * 512] = res.results[c]["y"]
    return out


# revision 42
# speedup vs baseline: 1.0166x; 1.0166x over previous
"""MLA (multi-head latent attention) Trainium2 kernel, 8-way tensor/data parallel.

Problem shapes (hardcoded): B=2, S=2048, HID=2048, H=16, KVH=4, DH=128, L=64.

Sharding: core c -> batch b = c//4, kv-group g = c%4.
Each core computes q-heads 4g..4g+3 and kv head g for its batch:
  - q/k/v projections (bf16 operands, f32 PSUM accum), RoPE
  - k-major attention (scoresT [Sk,Sq]) with no-max softmax, denominator via
    ones-matmul on PE, normalization via gpsimd partition-broadcast of 1/den
  - AllGather of per-head attnT (bf16) within the 4-core batch group
  - column-sharded output projection (each core owns 512 output columns)
  - latent cross-attention branch; per-kv-head contribution ReduceScattered so
    each core receives exactly its 512-column slice, folded into the output
    matmul as a rank-1 update.

Scheduling notes:
  - a dummy barrier collective at kernel start absorbs inter-core launch skew
    while the input DMAs stream, so the real collectives don't inherit it
  - hsT is staged seq-quarter-major so input DMAs move 4KB-contiguous rows
  - stage B runs two q-blocks in lockstep (interleaved per k-chunk) so each
    engine works on one stream while the other stream's cross-engine
    semaphore round-trip is in flight
  - stage C runs in two passes so the heads-0..2 contraction hides under the
    last AllGather; per-iteration serial tails never touch the Tensor queue.
Host side shards/prepares inputs and concatenates the 8 per-core outputs.
"""

import numpy as np
import ml_dtypes
from collections import deque
from contextlib import ExitStack

import concourse.bass as bass
import concourse.bacc as bacc
import concourse.tile as tile
import concourse.mybir as mybir
from concourse.bass_utils import run_bass_kernel_spmd

BF16 = ml_dtypes.bfloat16
FP32 = mybir.dt.float32
BF16_DT = mybir.dt.bfloat16

B, S, HID = 2, 2048, 2048
H, KVH, DH, L = 16, 4, 128, 64
THETA = 10000.0
N_CORES = 8
GROUPS = [[0, 1, 2, 3], [4, 5, 6, 7]]
NJ = HID // 128          # 16 contraction chunks
NSQ = S // 512           # 4 seq blocks of 512
NSB = S // 128           # 16 seq blocks of 128
QW = NJ * 512            # 8192 cols per seq-quarter in the sq-major hsT
SCALE = 1.0 / float(np.sqrt(np.float32(DH)))

_COMPILED = {}


def _emit_body(nc, tc, ctx, d, single_core):
    """Emit one full forward pass. d: dict of dram tensor handles."""
    # ---------- persistent pools ----------
    qk_pool = ctx.enter_context(tc.tile_pool(name="qk", bufs=1))
    v_pool = ctx.enter_context(tc.tile_pool(name="v", bufs=1))
    attn_pool = ctx.enter_context(tc.tile_pool(name="attn", bufs=1))
    const_pool = ctx.enter_context(tc.tile_pool(name="const", bufs=1))
    dram_pool = ctx.enter_context(tc.tile_pool(name="dram", bufs=1, space="DRAM"))

    # qT for 4 heads + kT (all roped, bf16):  [128, 5*S]
    qkT = qk_pool.tile([128, 5 * S], BF16_DT)
    # v in [s-part, dh] block layout: block sb -> [:, sb*128:(sb+1)*128]
    v_sb = v_pool.tile([128, S], BF16_DT)
    # normalized attnT for local 4 heads
    attnT = attn_pool.tile([128, 4 * S], BF16_DT)

    ones_col = const_pool.tile([128, 1], BF16_DT)     # den lhsT
    ones_row = const_pool.tile([1, 128], FP32)        # rank1 lhsT
    ident = const_pool.tile([128, 128], BF16_DT)
    shmat = const_pool.tile([128, 128], BF16_DT)      # rope half-swap permut.
    nc.vector.memset(ones_col[:], 1.0)
    nc.vector.memset(ones_row[:], 1.0)
    nc.sync.dma_start(ident[:], d["ident"].ap())
    nc.sync.dma_start(shmat[:], d["shmat"].ap())

    # DRAM bounce buffers for collectives (per-head AllGathers pipeline)
    bar_in = dram_pool.tile([1, 64], BF16_DT, name="bar_in")
    bar_out = dram_pool.tile([4, 64], BF16_DT, name="bar_out")
    ag_in = [dram_pool.tile([128, S], BF16_DT, tag=f"agi{h}", name=f"ag_in{h}")
             for h in range(4)]
    ag_out = [dram_pool.tile([512, S], BF16_DT, tag=f"ago{h}", name=f"ag_out{h}")
              for h in range(4)]
    rs_in = dram_pool.tile([1, HID], FP32)
    rs_out = dram_pool.tile([1, 512], FP32)

    # launch-skew barrier: enqueue first so it soaks up core start skew
    # while the input DMAs stream in
    if not single_core:
        nc.gpsimd.collective_compute(
            "AllGather", mybir.AluOpType.bypass, replica_groups=GROUPS,
            ins=[bar_in.opt()], outs=[bar_out.opt()])

    # ---------- stage A: projections + rope ----------
    with ExitStack() as actx:
        hs_pool = actx.enter_context(tc.tile_pool(name="hs", bufs=1))
        w_pool = actx.enter_context(tc.tile_pool(name="w", bufs=1))
        trig_pool = actx.enter_context(tc.tile_pool(name="trig", bufs=1))
        rope_pool = actx.enter_context(tc.tile_pool(name="rope", bufs=5))
        pA = actx.enter_context(tc.tile_pool(name="pA", bufs=2, space="PSUM"))
        pSh = actx.enter_context(tc.tile_pool(name="pSh", bufs=1, space="PSUM"))
        pT = actx.enter_context(tc.tile_pool(name="pT", bufs=1, space="PSUM"))

        # small tensors first so the first projections aren't DMA-starved
        wq_sb = w_pool.tile([128, NJ * 512], BF16_DT)
        wk_sb = w_pool.tile([128, NJ * 128], BF16_DT)
        wv_sb = w_pool.tile([128, NJ * 128], BF16_DT)
        nc.sync.dma_start(wk_sb[:], d["wk_sb"].ap())
        nc.sync.dma_start(wv_sb[:], d["wv_sb"].ap())

        # hsT staged seq-quarter-major: quarter sq occupies cols [sq*QW,
        # (sq+1)*QW) with contraction chunk j at [sq*QW + j*512, ...).
        # 4KB-contiguous rows per DMA descriptor. Quarter 0 loads before the
        # bulky wq so the first k-projection block starts ASAP.
        hsT = hs_pool.tile([128, NSQ * QW], BF16_DT)
        cosT = trig_pool.tile([128, S], FP32)
        sinS = trig_pool.tile([128, S], FP32)

        def hs_quarter(sq):
            for c in range(4):
                nc.sync.dma_start(
                    hsT[:, sq * QW + c * 2048: sq * QW + (c + 1) * 2048],
                    d["hsT"][sq][:, c * 2048:(c + 1) * 2048])

        hs_quarter(0)
        nc.sync.dma_start(cosT[:], d["cosT"].ap())
        nc.sync.dma_start(sinS[:], d["sinS"].ap())
        nc.sync.dma_start(wq_sb[:], d["wq_sb"].ap())
        for sq in range(1, NSQ):
            hs_quarter(sq)

        pending = deque()   # deferred per-block tails (emitted 1 block late)

        def drain(keep):
            while len(pending) > keep:
                pending.popleft()()

        def project_mm(w_sb, w_off, w_stride, sq):
            ps = pA.tile([128, 512], FP32, tag="proj")
            for j in range(NJ):
                nc.tensor.matmul(
                    ps[:],
                    lhsT=w_sb[:, w_off + j * w_stride:
                              w_off + j * w_stride + 128],
                    rhs=hsT[:, sq * QW + j * 512: sq * QW + j * 512 + 512],
                    start=(j == 0), stop=(j == NJ - 1),
                )
            return ps

        def rope_tail(ps, dst, dst_off, sq):
            def tail():
                # rope:  out = ps*cos + shift(ps)*sinS  (sign folded into sinS)
                # the partition half-swap runs on the PE via a permutation
                # matrix so no DMA sits in the chain
                qf = rope_pool.tile([128, 512], BF16_DT, tag="qf")
                nc.scalar.copy(qf[:], ps[:])
                qs = pSh.tile([128, 512], FP32, tag="qs")
                nc.tensor.matmul(qs[:], lhsT=shmat[:], rhs=qf[:],
                                 start=True, stop=True)
                m1 = rope_pool.tile([128, 512], FP32, tag="m1")
                nc.vector.tensor_mul(m1[:], ps[:], cosT[:, bass.ts(sq, 512)])
                m2 = rope_pool.tile([128, 512], FP32, tag="m2")
                nc.vector.tensor_mul(m2[:], qs[:], sinS[:, bass.ts(sq, 512)])
                nc.gpsimd.tensor_add(
                    dst[:, dst_off + sq * 512: dst_off + sq * 512 + 512],
                    m1[:], m2[:])
            return tail

        def v_tail(ps, sq):
            def tail():
                vT_bf = rope_pool.tile([128, 512], BF16_DT, tag="vbf")
                nc.scalar.copy(vT_bf[:], ps[:])
                tp = pT.tile([128, 512], BF16_DT, tag="tp")
                for i in range(4):
                    nc.tensor.transpose(
                        tp[:, bass.ts(i, 128)], vT_bf[:, bass.ts(i, 128)],
                        ident[:])
                nc.vector.tensor_copy(v_sb[:, bass.ts(sq, 512)], tp[:])
            return tail

        # k first, then v, then one q head; latent overlaps remaining q heads
        for sq in range(NSQ):
            ps = project_mm(wk_sb, 0, 128, sq)
            pending.append(rope_tail(ps, qkT, 4 * S, sq))
            drain(1)
        for sq in range(NSQ):
            ps = project_mm(wv_sb, 0, 128, sq)
            pending.append(v_tail(ps, sq))
            drain(1)
        for sq in range(NSQ):
            ps = project_mm(wq_sb, 0, 512, sq)
            pending.append(rope_tail(ps, qkT, 0, sq))
            drain(1)
        drain(0)

        # ---------- latent branch (tiny; overlaps q projections) ----------
        with ExitStack() as lctx:
            l_pool = lctx.enter_context(tc.tile_pool(name="lat", bufs=1))
            pLs = lctx.enter_context(tc.tile_pool(name="pLs", bufs=2, space="PSUM"))
            pLa = lctx.enter_context(tc.tile_pool(name="pLa", bufs=1, space="PSUM"))

            lat0 = l_pool.tile([128, L], BF16_DT)
            wlat = l_pool.tile([128, 128], BF16_DT)
            wlo = l_pool.tile([128, HID], FP32)
            nc.sync.dma_start(lat0[:], d["lat0T"].ap())
            nc.sync.dma_start(wlat[:], d["w_lat"].ap())
            nc.sync.dma_start(wlo[:], d["wlo"].ap())

            lp_ps = pLs.tile([128, L], FP32, tag="scL")
            nc.tensor.matmul(lp_ps[:], lhsT=wlat[:], rhs=lat0[:],
                             start=True, stop=True)
            latpT = l_pool.tile([128, L], BF16_DT)
            nc.scalar.copy(latpT[:], lp_ps[:])

            avL = pLa.tile([128, L], FP32, tag="avL")
            denL = pLa.tile([1, L], FP32, tag="denL")
            for kc4 in range(4):
                scL = pLs.tile([128, 4 * L], FP32, tag="scL")
                for i in range(4):
                    kc = kc4 * 4 + i
                    nc.tensor.matmul(
                        scL[:, bass.ts(i, L)],
                        lhsT=qkT[:, 4 * S + kc * 128: 4 * S + kc * 128 + 128],
                        rhs=latpT[:], start=True, stop=True)
                exL = l_pool.tile([128, 4 * L], BF16_DT, tag="exL")
                nc.scalar.activation(exL[:], scL[:],
                                     mybir.ActivationFunctionType.Exp,
                                     scale=SCALE)
                for i in range(4):
                    kc = kc4 * 4 + i
                    nc.tensor.matmul(avL[:], lhsT=v_sb[:, bass.ts(kc, 128)],
                                     rhs=exL[:, bass.ts(i, L)],
                                     start=(kc == 0), stop=(kc == NSB - 1))
                    nc.tensor.matmul(denL[:], lhsT=ones_col[:],
                                     rhs=exL[:, bass.ts(i, L)],
                                     start=(kc == 0), stop=(kc == NSB - 1))
            recipL = l_pool.tile([1, L], FP32, tag="recipL")
            nc.vector.reciprocal_approx_fast(recipL[:], denL[:])
            bcL_sb = l_pool.tile([128, L], FP32, tag="bcLsb")
            nc.gpsimd.partition_broadcast(bcL_sb[:], recipL[:])
            attLn = l_pool.tile([128, L], FP32, tag="attLn")
            nc.vector.tensor_mul(attLn[:], avL[:], bcL_sb[:])
            # mean over L then project through wlo rows of this kv head
            meanv = l_pool.tile([128, 1], FP32, tag="meanv")
            nc.vector.tensor_reduce(meanv[:], attLn[:],
                                    axis=mybir.AxisListType.X,
                                    op=mybir.AluOpType.add)
            nc.vector.tensor_scalar_mul(meanv[:], meanv[:], 1.0 / L)
            contrib_sb = l_pool.tile([1, HID], FP32, tag="contrib")
            for i in range(4):
                cps = pLs.tile([1, 512], FP32, tag="scL")
                nc.tensor.matmul(cps[:], lhsT=meanv[:],
                                 rhs=wlo[:, bass.ts(i, 512)],
                                 start=True, stop=True)
                nc.vector.tensor_copy(contrib_sb[:, bass.ts(i, 512)], cps[:])
            nc.sync.dma_start(rs_in[:], contrib_sb[:])

        if single_core:
            nc.sync.dma_start(rs_out[:], rs_in[:, 0:512])
        else:
            nc.gpsimd.collective_compute(
                "ReduceScatter", mybir.AluOpType.add, replica_groups=GROUPS,
                ins=[rs_in.opt()], outs=[rs_out.opt()])

        for hh in range(1, 4):
            for sq in range(NSQ):
                ps = project_mm(wq_sb, hh * 128, 512, sq)
                pending.append(rope_tail(ps, qkT, hh * S, sq))
                drain(1)
        drain(0)

    # ---------- stage B + C SBUF pools (hsT freed above) ----------
    with ExitStack() as sctx:
        e_pool = sctx.enter_context(tc.tile_pool(name="expd", bufs=8))
        n_pool = sctx.enter_context(tc.tile_pool(name="norm", bufs=3))
        g_pool = sctx.enter_context(tc.tile_pool(name="gath", bufs=1))
        wo_pool = sctx.enter_context(tc.tile_pool(name="wo", bufs=1))
        oa_pool = sctx.enter_context(tc.tile_pool(name="oacc", bufs=1))
        o_pool = sctx.enter_context(tc.tile_pool(name="oev", bufs=6))

        wo_sb = wo_pool.tile([128, NJ * 512], BF16_DT)
        nc.sync.dma_start(wo_sb[:], d["wo_sb"].ap())
        latrow = wo_pool.tile([1, 512], FP32)
        nc.sync.dma_start(latrow[:], rs_out[:])
        gath = g_pool.tile([128, NJ * S], BF16_DT)
        oacc = oa_pool.tile([128, NSB * 512], BF16_DT)

        # ---------- stage B: attention (k-major, pipelined chunk stream) -----
        # Flat stream of 8 chunk-pairs per (head, q-block) iteration with the
        # score matmuls emitted one chunk ahead of the av/den matmuls, so the
        # exp chain on Scalar never waits for a post-exp Tensor round-trip.
        with ExitStack() as bctx:
            pS = bctx.enter_context(tc.tile_pool(name="pS", bufs=2, space="PSUM"))
            pAV = bctx.enter_context(tc.tile_pool(name="pAV", bufs=2, space="PSUM"))
            pDen = bctx.enter_context(tc.tile_pool(name="pDen", bufs=2, space="PSUM"))

            def attn_tail(hh, sq, av, den):
                recip = n_pool.tile([1, 512], FP32, tag="recip")
                nc.vector.reciprocal_approx_fast(recip[:], den[:])
                bc_sb = n_pool.tile([128, 512], FP32, tag="bcsb")
                nc.gpsimd.partition_broadcast(bc_sb[:], recip[:])
                nc.vector.tensor_mul(
                    attnT[:, hh * S + sq * 512: hh * S + sq * 512 + 512],
                    av[:], bc_sb[:])
                if sq == NSQ - 1:
                    # ship this head's attnT out, gather it across the
                    # group, and pull the gathered rows back into SBUF
                    nc.sync.dma_start(ag_in[hh][:],
                                      attnT[:, hh * S:(hh + 1) * S])
                    if single_core:
                        nc.sync.dma_start(ag_out[hh][0:128, :], ag_in[hh][:])
                    else:
                        nc.gpsimd.collective_compute(
                            "AllGather", mybir.AluOpType.bypass,
                            replica_groups=GROUPS,
                            ins=[ag_in[hh].opt()], outs=[ag_out[hh].opt()])
                    for r in range(4):
                        j = 4 * r + hh
                        nc.sync.dma_start(
                            gath[:, bass.ts(j, S)],
                            ag_out[hh][r * 128:(r + 1) * 128, :])

            NCH = 16 * 8        # chunk-pairs total (16 iterations x 8)

            def sc_emit(n):
                it, c = divmod(n, 8)
                hh, sq = divmod(it, 4)
                off = hh * S + sq * 512
                sc = pS.tile([128, 1024], FP32, tag="sc")
                for i in range(2):
                    kc = c * 2 + i
                    nc.tensor.matmul(
                        sc[:, bass.ts(i, 512)],
                        lhsT=qkT[:, 4 * S + kc * 128: 4 * S + kc * 128 + 128],
                        rhs=qkT[:, off: off + 512],
                        start=True, stop=True)
                return sc

            scq = deque([sc_emit(0), sc_emit(1)])
            av = den = None
            for n in range(NCH):
                it, c = divmod(n, 8)
                hh, sq = divmod(it, 4)
                if c == 0:
                    av = pAV.tile([128, 512], FP32, tag="av")
                    den = pDen.tile([1, 512], FP32, tag="den")
                sc = scq.popleft()
                ex = e_pool.tile([128, 1024], BF16_DT, tag="ex")
                nc.scalar.activation(ex[:], sc[:],
                                     mybir.ActivationFunctionType.Exp,
                                     scale=SCALE)
                if n + 2 < NCH:
                    scq.append(sc_emit(n + 2))
                for i in range(2):
                    kc = c * 2 + i
                    nc.tensor.matmul(
                        av[:], lhsT=v_sb[:, bass.ts(kc, 128)],
                        rhs=ex[:, bass.ts(i, 512)],
                        start=(kc == 0), stop=(kc == NSB - 1))
                    nc.tensor.matmul(
                        den[:], lhsT=ones_col[:],
                        rhs=ex[:, bass.ts(i, 512)],
                        start=(kc == 0), stop=(kc == NSB - 1))
                if c == 7:
                    attn_tail(hh, sq, av, den)

        # ---------- stage C: output projection (two passes) ----------
        # pass 1: heads 0..2 contraction, runs under the AG3 collective wait
        with ExitStack() as cctx:
            pC = cctx.enter_context(tc.tile_pool(name="pC", bufs=4, space="PSUM"))

            # latent row folded into the pass-1 accumulator via one broadcast
            latbc = n_pool.tile([128, 512], FP32, tag="latbc")
            nc.gpsimd.partition_broadcast(latbc[:], latrow[:])

            J012 = [j for j in range(NJ) if j % 4 != 3]
            J3 = [j for j in range(NJ) if j % 4 == 3]
            ctails = deque()
            for sb in range(NSB):
                ps1 = pC.tile([128, 512], FP32, tag="c1")
                for idx, j in enumerate(J012):
                    nc.tensor.matmul(
                        ps1[:],
                        lhsT=gath[:, j * S + sb * 128: j * S + sb * 128 + 128],
                        rhs=wo_sb[:, bass.ts(j, 512)],
                        start=(idx == 0), stop=(idx == len(J012) - 1))

                def c1_tail(sb=sb, ps1=ps1):
                    nc.vector.tensor_add(oacc[:, bass.ts(sb, 512)], ps1[:],
                                         latbc[:])
                ctails.append(c1_tail)
                while len(ctails) > 1:
                    ctails.popleft()()
            while ctails:
                ctails.popleft()()

            # pass 2: last head's chunks + accumulated pass-1 + latent rank-1
            for sb in range(NSB):
                ps2 = pC.tile([128, 512], FP32, tag="c1")
                for idx, j in enumerate(J3):
                    nc.tensor.matmul(
                        ps2[:],
                        lhsT=gath[:, j * S + sb * 128: j * S + sb * 128 + 128],
                        rhs=wo_sb[:, bass.ts(j, 512)],
                        start=(idx == 0), stop=False)
                nc.tensor.matmul(ps2[:], lhsT=ident[:],
                                 rhs=oacc[:, bass.ts(sb, 512)],
                                 start=False, stop=True)

                def c2_tail(sb=sb, ps2=ps2):
                    oev = o_pool.tile([128, 512], FP32, tag="oev")
                    nc.vector.tensor_copy(oev[:], ps2[:])
                    nc.sync.dma_start(d["y"].ap()[sb * 128:(sb + 1) * 128, :],
                                      oev[:])
                ctails.append(c2_tail)
                while len(ctails) > 1:
                    ctails.popleft()()
            while ctails:
                ctails.popleft()()


def _build_kernel(reps=1, single_core=False):
    nc = bacc.Bacc("TRN2", target_bir_lowering=False, debug=False,
                   num_devices=(1 if single_core else N_CORES))

    d = {
        "hsT": nc.dram_tensor("hsT", [NSQ, 128, QW], BF16_DT, kind="ExternalInput"),
        "wq_sb": nc.dram_tensor("wq_sb", [128, NJ * 512], BF16_DT, kind="ExternalInput"),
        "wk_sb": nc.dram_tensor("wk_sb", [128, NJ * 128], BF16_DT, kind="ExternalInput"),
        "wv_sb": nc.dram_tensor("wv_sb", [128, NJ * 128], BF16_DT, kind="ExternalInput"),
        "wo_sb": nc.dram_tensor("wo_sb", [128, NJ * 512], BF16_DT, kind="ExternalInput"),
        "cosT": nc.dram_tensor("cosT", [128, S], FP32, kind="ExternalInput"),
        "sinS": nc.dram_tensor("sinS", [128, S], FP32, kind="ExternalInput"),
        "ident": nc.dram_tensor("ident", [128, 128], BF16_DT, kind="ExternalInput"),
        "shmat": nc.dram_tensor("shmat", [128, 128], BF16_DT, kind="ExternalInput"),
        "lat0T": nc.dram_tensor("lat0T", [128, L], BF16_DT, kind="ExternalInput"),
        "w_lat": nc.dram_tensor("w_lat", [128, 128], BF16_DT, kind="ExternalInput"),
        "wlo": nc.dram_tensor("wlo", [128, HID], FP32, kind="ExternalInput"),
        "y": nc.dram_tensor("y", [S, 512], FP32, kind="ExternalOutput"),
    }

    with tile.TileContext(nc) as tc:
        if reps == 1:
            with ExitStack() as ctx:
                _emit_body(nc, tc, ctx, d, single_core)
        else:
            with tc.For_i(0, reps, 1):
                with ExitStack() as ctx:
                    _emit_body(nc, tc, ctx, d, single_core)

    nc.compile()
    return nc


def _host_inputs(hs, latent, w_latent, wq, wk, wv, wo, wlo):
    """Build the 8 per-core input maps."""
    inv_freq = 1.0 / (THETA ** (np.arange(0, DH, 2, dtype=np.float32) / DH))
    t = np.arange(S, dtype=np.float32)
    freqs = np.outer(t, inv_freq)
    emb = np.concatenate([freqs, freqs], axis=-1)          # [S, DH]
    cosT = np.ascontiguousarray(np.cos(emb).T.astype(np.float32))
    sinT = np.sin(emb).T.astype(np.float32)
    sinS = sinT.copy()
    sinS[:64] *= -1.0
    sinS = np.ascontiguousarray(sinS)
    ident = np.eye(128, dtype=BF16)
    shmat = np.ascontiguousarray(np.roll(np.eye(128), 64, axis=0)).astype(BF16)

    def chunked(w, cols):
        # [HID, cols] -> [128, NJ*cols] with chunk-major free dim
        return np.ascontiguousarray(
            w.reshape(NJ, 128, cols).transpose(1, 0, 2).reshape(128, NJ * cols)
        ).astype(BF16)

    in_maps = []
    for c in range(N_CORES):
        b, g = c // 4, c % 4
        # [S, HID] -> [NSQ, 128(hid chunk row), NJ*512] seq-quarter-major
        hsT_full = np.ascontiguousarray(hs[b].T).astype(BF16)   # [HID, S]
        hsT = np.empty((NSQ, 128, QW), dtype=BF16)
        for sq in range(NSQ):
            for j in range(NJ):
                hsT[sq, :, j * 512:(j + 1) * 512] = \
                    hsT_full[j * 128:(j + 1) * 128, sq * 512:(sq + 1) * 512]
        in_maps.append({
            "hsT": hsT,
            "wq_sb": chunked(wq[:, 4 * g * DH:(4 * g + 4) * DH], 512),
            "wk_sb": chunked(wk[:, g * DH:(g + 1) * DH], 128),
            "wv_sb": chunked(wv[:, g * DH:(g + 1) * DH], 128),
            "wo_sb": chunked(wo[:, g * 512:(g + 1) * 512], 512),
            "cosT": cosT,
            "sinS": sinS,
            "ident": ident,
            "shmat": shmat,
            "lat0T": np.ascontiguousarray(latent[0, g].T).astype(BF16),
            "w_lat": w_latent.astype(BF16),
            "wlo": np.ascontiguousarray(wlo[g * DH:(g + 1) * DH, :]).astype(np.float32),
        })
    return in_maps


def kernel(hidden_states, latent, w_latent, wq, wk, wv, wo, w_latent_o,
           *, _trace=False, _trace_cores=None):
    hs = np.asarray(hidden_states, np.float32)
    in_maps = _host_inputs(hs, np.asarray(latent), np.asarray(w_latent),
                           np.asarray(wq), np.asarray(wk), np.asarray(wv),
                           np.asarray(wo), np.asarray(w_latent_o))
    if "nc" not in _COMPILED:
        _COMPILED["nc"] = _build_kernel()
    nc = _COMPILED["nc"]
    res = run_bass_kernel_spmd(nc, in_maps, list(range(N_CORES)),
                               trace=_trace, trace_cores=_trace_cores)
    kernel.last_result = res
    out = np.empty((B, S, HID), np.float32)
    for c in range(N_CORES):
        b, g = c // 4, c % 4
        out[b, :, g * 512:(g + 1) * 512] = res.results[c]["y"]
    return out


# revision 44
# speedup vs baseline: 1.0784x; 1.0608x over previous
"""MLA (multi-head latent attention) Trainium2 kernel, 8-way tensor/data parallel.

Problem shapes (hardcoded): B=2, S=2048, HID=2048, H=16, KVH=4, DH=128, L=64.

Sharding: core c -> batch b = c//4, kv-group g = c%4.
Each core computes q-heads 4g..4g+3 and kv head g for its batch:
  - q/k/v projections (bf16 operands, f32 PSUM accum), RoPE
  - k-major attention (scoresT [Sk,Sq]) with no-max softmax, denominator via
    ones-matmul on PE, normalization via gpsimd partition-broadcast of 1/den
  - AllGather of per-head attnT (bf16) within the 4-core batch group
  - column-sharded output projection (each core owns 512 output columns)
  - latent cross-attention branch; per-kv-head contribution ReduceScattered so
    each core receives exactly its 512-column slice, folded into the output
    matmul as a rank-1 update.

Scheduling notes:
  - a dummy barrier collective at kernel start absorbs inter-core launch skew
    while the input DMAs stream, so the real collectives don't inherit it
  - hsT is staged seq-quarter-major so input DMAs move 4KB-contiguous rows
  - stage B runs two q-blocks in lockstep (interleaved per k-chunk) so each
    engine works on one stream while the other stream's cross-engine
    semaphore round-trip is in flight
  - stage C runs in two passes so the heads-0..2 contraction hides under the
    last AllGather; per-iteration serial tails never touch the Tensor queue.
Host side shards/prepares inputs and concatenates the 8 per-core outputs.
"""

import numpy as np
import ml_dtypes
from collections import deque
from contextlib import ExitStack

import concourse.bass as bass
import concourse.bacc as bacc
import concourse.tile as tile
import concourse.mybir as mybir
from concourse.bass_utils import run_bass_kernel_spmd

BF16 = ml_dtypes.bfloat16
FP32 = mybir.dt.float32
BF16_DT = mybir.dt.bfloat16

B, S, HID = 2, 2048, 2048
H, KVH, DH, L = 16, 4, 128, 64
THETA = 10000.0
N_CORES = 8
GROUPS = [[0, 1, 2, 3], [4, 5, 6, 7]]
NJ = HID // 128          # 16 contraction chunks
NSQ = S // 512           # 4 seq blocks of 512
NSB = S // 128           # 16 seq blocks of 128
QW = NJ * 512            # 8192 cols per seq-quarter in the sq-major hsT
SCALE = 1.0 / float(np.sqrt(np.float32(DH)))

_COMPILED = {}


def _emit_body(nc, tc, ctx, d, single_core):
    """Emit one full forward pass. d: dict of dram tensor handles."""
    # ---------- persistent pools ----------
    qk_pool = ctx.enter_context(tc.tile_pool(name="qk", bufs=1))
    v_pool = ctx.enter_context(tc.tile_pool(name="v", bufs=1))
    attn_pool = ctx.enter_context(tc.tile_pool(name="attn", bufs=1))
    const_pool = ctx.enter_context(tc.tile_pool(name="const", bufs=1))
    dram_pool = ctx.enter_context(tc.tile_pool(name="dram", bufs=1, space="DRAM"))

    # qT for 4 heads + kT (all roped, bf16):  [128, 5*S]
    qkT = qk_pool.tile([128, 5 * S], BF16_DT)
    # v in [s-part, dh] block layout: block sb -> [:, sb*128:(sb+1)*128]
    v_sb = v_pool.tile([128, S], BF16_DT)
    # normalized attnT for local 4 heads
    attnT = attn_pool.tile([128, 4 * S], BF16_DT)

    ones_col = const_pool.tile([128, 1], BF16_DT)     # den lhsT
    ones_row = const_pool.tile([1, 128], FP32)        # rank1 lhsT
    ident = const_pool.tile([128, 128], BF16_DT)
    shmat = const_pool.tile([128, 128], BF16_DT)      # rope half-swap permut.
    nc.vector.memset(ones_col[:], 1.0)
    nc.vector.memset(ones_row[:], 1.0)
    nc.sync.dma_start(ident[:], d["ident"].ap())
    nc.sync.dma_start(shmat[:], d["shmat"].ap())

    # DRAM bounce buffers for collectives (per-head AllGathers pipeline)
    bar_in = dram_pool.tile([1, 64], BF16_DT, name="bar_in")
    bar_out = dram_pool.tile([4, 64], BF16_DT, name="bar_out")
    ag_in = [dram_pool.tile([128, S], BF16_DT, tag=f"agi{h}", name=f"ag_in{h}")
             for h in range(4)]
    ag_out = [dram_pool.tile([512, S], BF16_DT, tag=f"ago{h}", name=f"ag_out{h}")
              for h in range(4)]
    rs_in = dram_pool.tile([1, HID], FP32)
    rs_out = dram_pool.tile([1, 512], FP32)

    # launch-skew barrier: enqueue first so it soaks up core start skew
    # while the input DMAs stream in
    if not single_core:
        nc.gpsimd.collective_compute(
            "AllGather", mybir.AluOpType.bypass, replica_groups=GROUPS,
            ins=[bar_in.opt()], outs=[bar_out.opt()])

    # ---------- stage A: projections + rope ----------
    with ExitStack() as actx:
        hs_pool = actx.enter_context(tc.tile_pool(name="hs", bufs=1))
        w_pool = actx.enter_context(tc.tile_pool(name="w", bufs=1))
        trig_pool = actx.enter_context(tc.tile_pool(name="trig", bufs=1))
        rope_pool = actx.enter_context(tc.tile_pool(name="rope", bufs=5))
        pA = actx.enter_context(tc.tile_pool(name="pA", bufs=2, space="PSUM"))
        pSh = actx.enter_context(tc.tile_pool(name="pSh", bufs=1, space="PSUM"))
        pT = actx.enter_context(tc.tile_pool(name="pT", bufs=1, space="PSUM"))

        # small tensors first so the first projections aren't DMA-starved
        wq_sb = w_pool.tile([128, NJ * 512], BF16_DT)
        wk_sb = w_pool.tile([128, NJ * 128], BF16_DT)
        wv_sb = w_pool.tile([128, NJ * 128], BF16_DT)
        nc.sync.dma_start(wk_sb[:], d["wk_sb"].ap())
        nc.sync.dma_start(wv_sb[:], d["wv_sb"].ap())
        nc.sync.dma_start(wq_sb[:], d["wq_sb"].ap())

        cosT = trig_pool.tile([128, S], FP32)
        sinS = trig_pool.tile([128, S], FP32)
        nc.sync.dma_start(cosT[:], d["cosT"].ap())
        nc.sync.dma_start(sinS[:], d["sinS"].ap())

        # hsT staged seq-quarter-major: quarter sq occupies cols [sq*QW,
        # (sq+1)*QW) with contraction chunk j at [sq*QW + j*512, ...).
        # 4KB-contiguous rows per DMA descriptor.
        hsT = hs_pool.tile([128, NSQ * QW], BF16_DT)
        for sq in range(NSQ):
            for c in range(4):
                nc.sync.dma_start(
                    hsT[:, sq * QW + c * 2048: sq * QW + (c + 1) * 2048],
                    d["hsT"][sq][:, c * 2048:(c + 1) * 2048])

        pending = deque()   # deferred per-block tails (emitted 1 block late)

        def drain(keep):
            while len(pending) > keep:
                pending.popleft()()

        def project_mm(w_sb, w_off, w_stride, sq):
            ps = pA.tile([128, 512], FP32, tag="proj")
            for j in range(NJ):
                nc.tensor.matmul(
                    ps[:],
                    lhsT=w_sb[:, w_off + j * w_stride:
                              w_off + j * w_stride + 128],
                    rhs=hsT[:, sq * QW + j * 512: sq * QW + j * 512 + 512],
                    start=(j == 0), stop=(j == NJ - 1),
                )
            return ps

        def rope_tail(ps, dst, dst_off, sq):
            def tail():
                # rope:  out = ps*cos + shift(ps)*sinS  (sign folded into sinS)
                # the partition half-swap runs on the PE via a permutation
                # matrix so no DMA sits in the chain
                qf = rope_pool.tile([128, 512], BF16_DT, tag="qf")
                nc.scalar.copy(qf[:], ps[:])
                qs = pSh.tile([128, 512], FP32, tag="qs")
                nc.tensor.matmul(qs[:], lhsT=shmat[:], rhs=qf[:],
                                 start=True, stop=True)
                m1 = rope_pool.tile([128, 512], FP32, tag="m1")
                nc.vector.tensor_mul(m1[:], ps[:], cosT[:, bass.ts(sq, 512)])
                m2 = rope_pool.tile([128, 512], FP32, tag="m2")
                nc.vector.tensor_mul(m2[:], qs[:], sinS[:, bass.ts(sq, 512)])
                nc.gpsimd.tensor_add(
                    dst[:, dst_off + sq * 512: dst_off + sq * 512 + 512],
                    m1[:], m2[:])
            return tail

        def v_tail(ps, sq):
            def tail():
                vT_bf = rope_pool.tile([128, 512], BF16_DT, tag="vbf")
                nc.scalar.copy(vT_bf[:], ps[:])
                tp = pT.tile([128, 512], BF16_DT, tag="tp")
                for i in range(4):
                    nc.tensor.transpose(
                        tp[:, bass.ts(i, 128)], vT_bf[:, bass.ts(i, 128)],
                        ident[:])
                nc.vector.tensor_copy(v_sb[:, bass.ts(sq, 512)], tp[:])
            return tail

        # k first, then v, then one q head; latent overlaps remaining q heads
        for sq in range(NSQ):
            ps = project_mm(wk_sb, 0, 128, sq)
            pending.append(rope_tail(ps, qkT, 4 * S, sq))
            drain(1)
        for sq in range(NSQ):
            ps = project_mm(wv_sb, 0, 128, sq)
            pending.append(v_tail(ps, sq))
            drain(1)
        for sq in range(NSQ):
            ps = project_mm(wq_sb, 0, 512, sq)
            pending.append(rope_tail(ps, qkT, 0, sq))
            drain(1)
        drain(0)

        # ---------- latent branch (tiny; overlaps q projections) ----------
        with ExitStack() as lctx:
            l_pool = lctx.enter_context(tc.tile_pool(name="lat", bufs=1))
            pLs = lctx.enter_context(tc.tile_pool(name="pLs", bufs=2, space="PSUM"))
            pLa = lctx.enter_context(tc.tile_pool(name="pLa", bufs=1, space="PSUM"))

            lat0 = l_pool.tile([128, L], BF16_DT)
            wlat = l_pool.tile([128, 128], BF16_DT)
            wlo = l_pool.tile([128, HID], FP32)
            nc.sync.dma_start(lat0[:], d["lat0T"].ap())
            nc.sync.dma_start(wlat[:], d["w_lat"].ap())
            nc.sync.dma_start(wlo[:], d["wlo"].ap())

            lp_ps = pLs.tile([128, L], FP32, tag="scL")
            nc.tensor.matmul(lp_ps[:], lhsT=wlat[:], rhs=lat0[:],
                             start=True, stop=True)
            latpT = l_pool.tile([128, L], BF16_DT)
            nc.scalar.copy(latpT[:], lp_ps[:])

            avL = pLa.tile([128, L], FP32, tag="avL")
            denL = pLa.tile([1, L], FP32, tag="denL")
            for kc4 in range(4):
                scL = pLs.tile([128, 4 * L], FP32, tag="scL")
                for i in range(4):
                    kc = kc4 * 4 + i
                    nc.tensor.matmul(
                        scL[:, bass.ts(i, L)],
                        lhsT=qkT[:, 4 * S + kc * 128: 4 * S + kc * 128 + 128],
                        rhs=latpT[:], start=True, stop=True)
                exL = l_pool.tile([128, 4 * L], BF16_DT, tag="exL")
                nc.scalar.activation(exL[:], scL[:],
                                     mybir.ActivationFunctionType.Exp,
                                     scale=SCALE)
                for i in range(4):
                    kc = kc4 * 4 + i
                    nc.tensor.matmul(avL[:], lhsT=v_sb[:, bass.ts(kc, 128)],
                                     rhs=exL[:, bass.ts(i, L)],
                                     start=(kc == 0), stop=(kc == NSB - 1))
                    nc.tensor.matmul(denL[:], lhsT=ones_col[:],
                                     rhs=exL[:, bass.ts(i, L)],
                                     start=(kc == 0), stop=(kc == NSB - 1))
            recipL = l_pool.tile([1, L], FP32, tag="recipL")
            nc.vector.reciprocal_approx_fast(recipL[:], denL[:])
            bcL_sb = l_pool.tile([128, L], FP32, tag="bcLsb")
            nc.gpsimd.partition_broadcast(bcL_sb[:], recipL[:])
            attLn = l_pool.tile([128, L], FP32, tag="attLn")
            nc.vector.tensor_mul(attLn[:], avL[:], bcL_sb[:])
            # mean over L then project through wlo rows of this kv head
            meanv = l_pool.tile([128, 1], FP32, tag="meanv")
            nc.vector.tensor_reduce(meanv[:], attLn[:],
                                    axis=mybir.AxisListType.X,
                                    op=mybir.AluOpType.add)
            nc.vector.tensor_scalar_mul(meanv[:], meanv[:], 1.0 / L)
            contrib_sb = l_pool.tile([1, HID], FP32, tag="contrib")
            for i in range(4):
                cps = pLs.tile([1, 512], FP32, tag="scL")
                nc.tensor.matmul(cps[:], lhsT=meanv[:],
                                 rhs=wlo[:, bass.ts(i, 512)],
                                 start=True, stop=True)
                nc.vector.tensor_copy(contrib_sb[:, bass.ts(i, 512)], cps[:])
            nc.sync.dma_start(rs_in[:], contrib_sb[:])

        if single_core:
            nc.sync.dma_start(rs_out[:], rs_in[:, 0:512])
        else:
            nc.gpsimd.collective_compute(
                "ReduceScatter", mybir.AluOpType.add, replica_groups=GROUPS,
                ins=[rs_in.opt()], outs=[rs_out.opt()])

        for hh in range(1, 4):
            for sq in range(NSQ):
                ps = project_mm(wq_sb, hh * 128, 512, sq)
                pending.append(rope_tail(ps, qkT, hh * S, sq))
                drain(1)
        drain(0)

    # ---------- stage B + C SBUF pools (hsT freed above) ----------
    with ExitStack() as sctx:
        e_pool = sctx.enter_context(tc.tile_pool(name="expd", bufs=4))
        n_pool = sctx.enter_context(tc.tile_pool(name="norm", bufs=3))
        g_pool = sctx.enter_context(tc.tile_pool(name="gath", bufs=1))
        wo_pool = sctx.enter_context(tc.tile_pool(name="wo", bufs=1))
        oa_pool = sctx.enter_context(tc.tile_pool(name="oacc", bufs=1))
        o_pool = sctx.enter_context(tc.tile_pool(name="oev", bufs=6))

        wo_sb = wo_pool.tile([128, NJ * 512], BF16_DT)
        nc.sync.dma_start(wo_sb[:], d["wo_sb"].ap())
        latrow = wo_pool.tile([1, 512], FP32)
        nc.sync.dma_start(latrow[:], rs_out[:])
        gath = g_pool.tile([128, NJ * S], BF16_DT)
        oacc = oa_pool.tile([128, NSB * 512], BF16_DT)

        # ---------- stage B: attention (k-major, pipelined chunk stream) -----
        # Flat stream of 8 chunk-pairs per (head, q-block) iteration with the
        # score matmuls emitted one chunk ahead of the av/den matmuls, so the
        # exp chain on Scalar never waits for a post-exp Tensor round-trip.
        with ExitStack() as bctx:
            pS2 = bctx.enter_context(tc.tile_pool(name="pS2", bufs=1, space="PSUM"))
            pS1 = bctx.enter_context(tc.tile_pool(name="pS1", bufs=1, space="PSUM"))
            pAV = bctx.enter_context(tc.tile_pool(name="pAV", bufs=1, space="PSUM"))
            pDen = bctx.enter_context(tc.tile_pool(name="pDen", bufs=1, space="PSUM"))

            def attn_tail(hh, sq, av, den):
                recip = n_pool.tile([1, 512], FP32, tag="recip")
                nc.vector.reciprocal_approx_fast(recip[:], den[:])
                bc_sb = n_pool.tile([128, 512], FP32, tag="bcsb")
                nc.gpsimd.partition_broadcast(bc_sb[:], recip[:])
                nc.vector.tensor_mul(
                    attnT[:, hh * S + sq * 512: hh * S + sq * 512 + 512],
                    av[:], bc_sb[:])
                if sq == NSQ - 1:
                    # ship this head's attnT out, gather it across the
                    # group, and pull the gathered rows back into SBUF
                    nc.sync.dma_start(ag_in[hh][:],
                                      attnT[:, hh * S:(hh + 1) * S])
                    if single_core:
                        nc.sync.dma_start(ag_out[hh][0:128, :], ag_in[hh][:])
                    else:
                        nc.gpsimd.collective_compute(
                            "AllGather", mybir.AluOpType.bypass,
                            replica_groups=GROUPS,
                            ins=[ag_in[hh].opt()], outs=[ag_out[hh].opt()])
                    for r in range(4):
                        j = 4 * r + hh
                        nc.sync.dma_start(
                            gath[:, bass.ts(j, S)],
                            ag_out[hh][r * 128:(r + 1) * 128, :])

            # mixed-width chunk schedule: [4,2,4,2,4] k-chunks per
            # iteration, alternating between the two single-buffered pools
            # (same WAR timing as a double-buffered pool, fewer and wider
            # exp instructions -> less fixed ACT issue overhead)
            CHUNKS = [(0, 4), (4, 2), (6, 4), (10, 2), (12, 4)]
            ENTRIES = [(it, s0, nk) for it in range(16) for s0, nk in CHUNKS]

            def sc_emit(e):
                it, s0, nk = ENTRIES[e]
                hh, sq = divmod(it, 4)
                off = hh * S + sq * 512
                pool = pS2 if nk == 4 else pS1
                sc = pool.tile([128, nk * 512], FP32, tag="sc")
                for i in range(nk):
                    kc = s0 + i
                    nc.tensor.matmul(
                        sc[:, bass.ts(i, 512)],
                        lhsT=qkT[:, 4 * S + kc * 128: 4 * S + kc * 128 + 128],
                        rhs=qkT[:, off: off + 512],
                        start=True, stop=True)
                return sc

            scq = deque([sc_emit(0), sc_emit(1)])
            av = den = None
            for e, (it, s0, nk) in enumerate(ENTRIES):
                hh, sq = divmod(it, 4)
                if s0 == 0:
                    av = pAV.tile([128, 512], FP32, tag="av")
                    den = pDen.tile([1, 512], FP32, tag="den")
                sc = scq.popleft()
                ex = e_pool.tile([128, nk * 512], BF16_DT,
                                 tag=f"ex{nk}")
                nc.scalar.activation(ex[:], sc[:],
                                     mybir.ActivationFunctionType.Exp,
                                     scale=SCALE)
                if e + 2 < len(ENTRIES):
                    scq.append(sc_emit(e + 2))
                for i in range(nk):
                    kc = s0 + i
                    nc.tensor.matmul(
                        av[:], lhsT=v_sb[:, bass.ts(kc, 128)],
                        rhs=ex[:, bass.ts(i, 512)],
                        start=(kc == 0), stop=(kc == NSB - 1))
                    nc.tensor.matmul(
                        den[:], lhsT=ones_col[:],
                        rhs=ex[:, bass.ts(i, 512)],
                        start=(kc == 0), stop=(kc == NSB - 1))
                if s0 + nk == NSB:
                    attn_tail(hh, sq, av, den)

        # ---------- stage C: output projection (two passes) ----------
        # pass 1: heads 0..2 contraction, runs under the AG3 collective wait
        with ExitStack() as cctx:
            pC = cctx.enter_context(tc.tile_pool(name="pC", bufs=4, space="PSUM"))

            # latent row folded into the pass-1 accumulator via one broadcast
            latbc = n_pool.tile([128, 512], FP32, tag="latbc")
            nc.gpsimd.partition_broadcast(latbc[:], latrow[:])

            J012 = [j for j in range(NJ) if j % 4 != 3]
            J3 = [j for j in range(NJ) if j % 4 == 3]
            ctails = deque()
            for sb in range(NSB):
                ps1 = pC.tile([128, 512], FP32, tag="c1")
                for idx, j in enumerate(J012):
                    nc.tensor.matmul(
                        ps1[:],
                        lhsT=gath[:, j * S + sb * 128: j * S + sb * 128 + 128],
                        rhs=wo_sb[:, bass.ts(j, 512)],
                        start=(idx == 0), stop=(idx == len(J012) - 1))

                def c1_tail(sb=sb, ps1=ps1):
                    nc.vector.tensor_add(oacc[:, bass.ts(sb, 512)], ps1[:],
                                         latbc[:])
                ctails.append(c1_tail)
                while len(ctails) > 1:
                    ctails.popleft()()
            while ctails:
                ctails.popleft()()

            # pass 2: last head's chunks + accumulated pass-1 + latent rank-1
            for sb in range(NSB):
                ps2 = pC.tile([128, 512], FP32, tag="c1")
                for idx, j in enumerate(J3):
                    nc.tensor.matmul(
                        ps2[:],
                        lhsT=gath[:, j * S + sb * 128: j * S + sb * 128 + 128],
                        rhs=wo_sb[:, bass.ts(j, 512)],
                        start=(idx == 0), stop=False)
                nc.tensor.matmul(ps2[:], lhsT=ident[:],
                                 rhs=oacc[:, bass.ts(sb, 512)],
                                 start=False, stop=True)

                def c2_tail(sb=sb, ps2=ps2):
                    oev = o_pool.tile([128, 512], FP32, tag="oev")
                    nc.vector.tensor_copy(oev[:], ps2[:])
                    nc.sync.dma_start(d["y"].ap()[sb * 128:(sb + 1) * 128, :],
                                      oev[:])
                ctails.append(c2_tail)
                while len(ctails) > 1:
                    ctails.popleft()()
            while ctails:
                ctails.popleft()()


def _build_kernel(reps=1, single_core=False):
    nc = bacc.Bacc("TRN2", target_bir_lowering=False, debug=False,
                   num_devices=(1 if single_core else N_CORES))

    d = {
        "hsT": nc.dram_tensor("hsT", [NSQ, 128, QW], BF16_DT, kind="ExternalInput"),
        "wq_sb": nc.dram_tensor("wq_sb", [128, NJ * 512], BF16_DT, kind="ExternalInput"),
        "wk_sb": nc.dram_tensor("wk_sb", [128, NJ * 128], BF16_DT, kind="ExternalInput"),
        "wv_sb": nc.dram_tensor("wv_sb", [128, NJ * 128], BF16_DT, kind="ExternalInput"),
        "wo_sb": nc.dram_tensor("wo_sb", [128, NJ * 512], BF16_DT, kind="ExternalInput"),
        "cosT": nc.dram_tensor("cosT", [128, S], FP32, kind="ExternalInput"),
        "sinS": nc.dram_tensor("sinS", [128, S], FP32, kind="ExternalInput"),
        "ident": nc.dram_tensor("ident", [128, 128], BF16_DT, kind="ExternalInput"),
        "shmat": nc.dram_tensor("shmat", [128, 128], BF16_DT, kind="ExternalInput"),
        "lat0T": nc.dram_tensor("lat0T", [128, L], BF16_DT, kind="ExternalInput"),
        "w_lat": nc.dram_tensor("w_lat", [128, 128], BF16_DT, kind="ExternalInput"),
        "wlo": nc.dram_tensor("wlo", [128, HID], FP32, kind="ExternalInput"),
        "y": nc.dram_tensor("y", [S, 512], FP32, kind="ExternalOutput"),
    }

    with tile.TileContext(nc) as tc:
        if reps == 1:
            with ExitStack() as ctx:
                _emit_body(nc, tc, ctx, d, single_core)
        else:
            with tc.For_i(0, reps, 1):
                with ExitStack() as ctx:
                    _emit_body(nc, tc, ctx, d, single_core)

    nc.compile()
    return nc


def _host_inputs(hs, latent, w_latent, wq, wk, wv, wo, wlo):
    """Build the 8 per-core input maps."""
    inv_freq = 1.0 / (THETA ** (np.arange(0, DH, 2, dtype=np.float32) / DH))
    t = np.arange(S, dtype=np.float32)
    freqs = np.outer(t, inv_freq)
    emb = np.concatenate([freqs, freqs], axis=-1)          # [S, DH]
    cosT = np.ascontiguousarray(np.cos(emb).T.astype(np.float32))
    sinT = np.sin(emb).T.astype(np.float32)
    sinS = sinT.copy()
    sinS[:64] *= -1.0
    sinS = np.ascontiguousarray(sinS)
    ident = np.eye(128, dtype=BF16)
    shmat = np.ascontiguousarray(np.roll(np.eye(128), 64, axis=0)).astype(BF16)

    def chunked(w, cols):
        # [HID, cols] -> [128, NJ*cols] with chunk-major free dim
        return np.ascontiguousarray(
            w.reshape(NJ, 128, cols).transpose(1, 0, 2).reshape(128, NJ * cols)
        ).astype(BF16)

    in_maps = []
    for c in range(N_CORES):
        b, g = c // 4, c % 4
        # [S, HID] -> [NSQ, 128(hid chunk row), NJ*512] seq-quarter-major
        hsT_full = np.ascontiguousarray(hs[b].T).astype(BF16)   # [HID, S]
        hsT = np.empty((NSQ, 128, QW), dtype=BF16)
        for sq in range(NSQ):
            for j in range(NJ):
                hsT[sq, :, j * 512:(j + 1) * 512] = \
                    hsT_full[j * 128:(j + 1) * 128, sq * 512:(sq + 1) * 512]
        in_maps.append({
            "hsT": hsT,
            "wq_sb": chunked(wq[:, 4 * g * DH:(4 * g + 4) * DH], 512),
            "wk_sb": chunked(wk[:, g * DH:(g + 1) * DH], 128),
            "wv_sb": chunked(wv[:, g * DH:(g + 1) * DH], 128),
            "wo_sb": chunked(wo[:, g * 512:(g + 1) * 512], 512),
            "cosT": cosT,
            "sinS": sinS,
            "ident": ident,
            "shmat": shmat,
            "lat0T": np.ascontiguousarray(latent[0, g].T).astype(BF16),
            "w_lat": w_latent.astype(BF16),
            "wlo": np.ascontiguousarray(wlo[g * DH:(g + 1) * DH, :]).astype(np.float32),
        })
    return in_maps


def kernel(hidden_states, latent, w_latent, wq, wk, wv, wo, w_latent_o,
           *, _trace=False, _trace_cores=None):
    hs = np.asarray(hidden_states, np.float32)
    in_maps = _host_inputs(hs, np.asarray(latent), np.asarray(w_latent),
                           np.asarray(wq), np.asarray(wk), np.asarray(wv),
                           np.asarray(wo), np.asarray(w_latent_o))
    if "nc" not in _COMPILED:
        _COMPILED["nc"] = _build_kernel()
    nc = _COMPILED["nc"]
    res = run_bass_kernel_spmd(nc, in_maps, list(range(N_CORES)),
                               trace=_trace, trace_cores=_trace_cores)
    kernel.last_result = res
    out = np.empty((B, S, HID), np.float32)
    for c in range(N_CORES):
        b, g = c // 4, c % 4
        out[b, :, g * 512:(g + 1) * 512] = res.results[c]["y"]
    return out
